# revision 11
# baseline (speedup 1.0000x reference)
"""KACacheAttention Trainium2 kernel — 8-core tensor-parallel over heads.

Sharding: each of 8 cores owns 2 heads x 2 batches (4 (b,h) pairs).
QKV + RoPE + attention computed per-core in transposed layouts (fp32r
matmuls at full PE rate); AllToAll switches to token sharding for the
dense output projection (each core produces a 256-token slice).

Host side: pre-transposes hidden_states / cached_k, precomputes RoPE
cos/sin tables from position_ids, slices weights per head group, and
reassembles full outputs (concat only — all math on device).
"""
import sys
import numpy as np

sys.path.insert(0, "/opt/trn_rl_repo")

import concourse.bass as bass
import concourse.mybir as mybir
import concourse.tile as tile
from concourse import tile_sem_assignment as _tsa
from concourse import tile_utils as _tu

# single SWDGE/HWDGE sem lane: queue FIFO == sem order, keeps per-inst
# waits at 1 (this walrus build rejects >1 sync-wait per instruction)
_tsa.NUM_SWDGE_GLOBAL_SEMS = 1
_tsa.NUM_HWDGE_SEMS = 1
_tu.max_sbuf_usage = 206 * 1024

from concourse.bass_utils import run_bass_kernel_spmd

f32 = mybir.dt.float32
f32r = mybir.dt.float32r

NC = 8          # cores
B, S, H = 2, 1024, 2048
NH, HD, ROT = 16, 128, 32
CACHED = 1024
T = CACHED + S  # 2048 total key length
HPC = NH // NC  # 2 heads per core
BH = B * HPC    # 4 (b,h) pairs per core
TOK = B * S     # 2048 tokens (b-major)
TSL = TOK // NC  # 256 token slice per core after A2A
SCALE = HD ** -0.5
BASE = 10000.0
KB = H // 128   # 16 contraction blocks

_COMPILED = None


def _split_multiwaits(nc):
    """Walrus (this build) allows ONE sync-wait per instruction. Split
    multi-wait instructions by hoisting excess waits onto NoOps."""
    cnt = 0
    for f in nc.m.functions:
        for blk in f.blocks:
            out = []
            for ins in blk.instructions:
                si = ins.sync_info
                if si is not None and si.on_wait is not None and len(si.on_wait) > 1:
                    waits = list(si.on_wait)
                    for w in waits[:-1]:
                        cnt += 1
                        nop = mybir.InstNoOp(name=f"I-mw-{cnt}", engine=ins.engine)
                        nop.sync_info = mybir.SyncInfo(on_wait=[w], on_update=[])
                        out.append(nop)
                    ins.sync_info = mybir.SyncInfo(
                        on_wait=[waits[-1]], on_update=list(si.on_update or [])
                    )
                out.append(ins)
            blk.instructions = out
    return cnt


def _build():
    nc = bass.Bass(num_devices=NC)

    d_ht = nc.dram_tensor("ht", [H, TOK], f32, kind="ExternalInput")
    d_wqk = nc.dram_tensor("wqk", [H, 4 * HD], f32, kind="ExternalInput")
    d_wv = nc.dram_tensor("wv", [H, 2 * HD], f32, kind="ExternalInput")
    d_bqk = nc.dram_tensor("bqk", [HD, 4], f32, kind="ExternalInput")
    d_bv = nc.dram_tensor("bv", [1, 2 * HD], f32, kind="ExternalInput")
    d_kc = nc.dram_tensor("kc", [BH, HD, CACHED], f32, kind="ExternalInput")
    d_va = nc.dram_tensor("va", [BH, CACHED, HD], f32, kind="ExternalInput")
    d_wd = nc.dram_tensor("wd", [H, H], f32, kind="ExternalInput")
    d_bd = nc.dram_tensor("bd", [1, H], f32, kind="ExternalInput")
    d_cs = nc.dram_tensor("cs", [2 * ROT, TOK], f32, kind="ExternalInput")
    d_tri = nc.dram_tensor("tri", [HD, 4 * 512], f32, kind="ExternalInput")
    d_idn = nc.dram_tensor("idn", [HD, HD], f32, kind="ExternalInput")
    d_on128 = nc.dram_tensor("on128", [HD, 1], f32, kind="ExternalInput")
    d_on1 = nc.dram_tensor("on1", [1, HD], f32, kind="ExternalInput")

    d_knew = nc.dram_tensor("knew", [BH, S, HD], f32, kind="ExternalOutput")
    d_anew = nc.dram_tensor("anew", [BH, S, HD], f32, kind="ExternalOutput")
    d_out = nc.dram_tensor("outp", [TSL, H], f32, kind="ExternalOutput")

    with tile.TileContext(nc) as tc, nc.allow_low_precision(
            reason="fp32r operands for full-rate PE"):
        with (
            tc.tile_pool(name="consts", bufs=1) as pc,
            tc.tile_pool(name="persist", bufs=1) as pq,
            tc.tile_pool(name="dram", bufs=1, space="DRAM") as pdram,
        ):
            # ---- small consts (SWDGE queue head) ----
            on128 = pc.tile([HD, 1], f32r)
            nc.gpsimd.dma_start(on128[:], d_on128[:].bitcast(f32r))
            on1 = pc.tile([1, HD], f32r)
            nc.gpsimd.dma_start(on1[:], d_on1[:].bitcast(f32r))
            csc = pc.tile([ROT, TOK], f32r)
            nc.gpsimd.dma_start(csc[:], d_cs[:].bitcast(f32r)[0:ROT, :])
            css = pc.tile([ROT, TOK], f32r)
            nc.gpsimd.dma_start(css[:], d_cs[:].bitcast(f32r)[ROT:2 * ROT, :])
            bqk = pc.tile([HD, 4], f32)
            nc.gpsimd.dma_start(bqk[:], d_bqk[:])
            bv = pc.tile([1, 2 * HD], f32r)
            nc.gpsimd.dma_start(bv[:], d_bv[:].bitcast(f32r))

            # persistent per-core products
            qk = pq.tile([128, 4 * TOK], f32r)       # q0 q1 k0 k1 (transposed)
            vsb = pq.tile([128, 16 * 2 * HD], f32r)  # natural V: 16 tok-tiles x 256
            anT = pq.tile([128, BH * S], f32r)       # normalized attnT per bh

            with (
                tc.tile_pool(name="wv", bufs=1) as pwv,
                tc.tile_pool(name="wstream", bufs=2) as pw,
                tc.tile_pool(name="htq", bufs=2) as phtq,
            ):
                wv = pwv.tile([128, KB * 2 * HD], f32r)
                nc.gpsimd.dma_start(
                    wv[:].rearrange("p (kb m) -> p kb m", kb=KB),
                    d_wv[:].bitcast(f32r).rearrange("(kb p) m -> p kb m", p=128))

                # ---- QKV: stream hiddenT in 4 quarters (4 kb each),
                # accumulate partial sums in SBUF ----
                with tc.tile_pool(name="ps_qk", bufs=3, space="PSUM") as psqk:
                    for qp in range(4):           # quarter pass
                        htq = phtq.tile([128, 4 * TOK], f32r, tag="htq",
                                        name=f"htq{qp}")
                        nc.gpsimd.dma_start(
                            htq[:].rearrange("p (kb t) -> p kb t", kb=4),
                            d_ht[:].bitcast(f32r)[qp * 512:(qp + 1) * 512, :]
                            .rearrange("(kb p) t -> p kb t", p=128))
                        wqp = pw.tile([128, 4 * 512], f32r, tag="wstream",
                                      name=f"wqp{qp}")
                        nc.gpsimd.dma_start(
                            wqp[:].rearrange("p (kb m) -> p kb m", kb=4),
                            d_wqk[:].bitcast(f32r)[qp * 512:(qp + 1) * 512, :]
                            .rearrange("(kb p) m -> p kb m", p=128))
                        # QK
                        for ob in range(4):
                            for tb in range(4):
                                ps = psqk.tile([128, 512], f32, tag="psqk",
                                               name=f"psqk{qp}{ob}{tb}")
                                for kb in range(4):
                                    nc.tensor.matmul(
                                        ps[:],
                                        wqp[:, kb * 512 + ob * 128:
                                            kb * 512 + ob * 128 + 128],
                                        htq[:, kb * TOK + tb * 512:
                                            kb * TOK + tb * 512 + 512],
                                        start=(kb == 0), stop=(kb == 3))
                                dst = qk[:, ob * TOK + tb * 512:
                                         ob * TOK + tb * 512 + 512]
                                if qp == 0:
                                    nc.vector.tensor_scalar_add(
                                        dst, ps[:], bqk[:, ob:ob + 1])
                                else:
                                    nc.vector.tensor_add(dst, dst, ps[:])
                        # V
                        for vtb in range(16):
                            psv = psqk.tile([128, 2 * HD], f32, tag="psv",
                                            name=f"psv{qp}{vtb}")
                            for kb in range(4):
                                gkb = qp * 4 + kb
                                nc.tensor.matmul(
                                    psv[:],
                                    htq[:, kb * TOK + vtb * 128:
                                        kb * TOK + vtb * 128 + 128],
                                    wv[:, gkb * 2 * HD:(gkb + 1) * 2 * HD],
                                    start=(kb == 0),
                                    stop=(kb == 3 and qp != 3))
                            if qp == 3:
                                nc.tensor.matmul(psv[:], on1[:], bv[:],
                                                 start=False, stop=True)
                            dstv = vsb[:, vtb * 2 * HD:(vtb + 1) * 2 * HD]
                            if qp == 0:
                                nc.vector.tensor_copy(dstv, psv[:])
                            else:
                                nc.vector.tensor_add(dstv, dstv, psv[:])

                # -------- RoPE on q0 q1 k0 k1 (partitions 0:32) --------
                for ob in range(4):
                    tmp = pw.tile([32, TOK], f32r, tag="wstream", name=f"rt{ob}")
                    sl = qk[:, ob * TOK:(ob + 1) * TOK]
                    # rotate-half across partitions via SBUF->SBUF DMA
                    nc.sync.dma_start(tmp[0:16, :], sl[16:32, :])
                    nc.sync.dma_start(tmp[16:32, :], sl[0:16, :])
                    nc.vector.tensor_mul(tmp[:], tmp[:], css[:])
                    nc.vector.tensor_mul(sl[0:32, :], sl[0:32, :], csc[:])
                    nc.vector.tensor_add(sl[0:32, :], sl[0:32, :], tmp[:])

            # caches + late consts + wd stream (tail of SWDGE queue)
            with (
                tc.tile_pool(name="cache", bufs=1) as pcache,
                tc.tile_pool(name="lconst", bufs=1) as plc,
                tc.tile_pool(name="wd", bufs=4) as pwd,
                tc.tile_pool(name="atall", bufs=1) as pat,
            ):
                kcache = pcache.tile([128, BH * CACHED], f32r)
                nc.gpsimd.dma_start(
                    kcache[:].rearrange("p (bh t) -> p bh t", bh=BH),
                    d_kc[:].bitcast(f32r).rearrange("bh p t -> p bh t"))
                va = pcache.tile([128, BH * 8 * HD], f32r)
                nc.gpsimd.dma_start(
                    va[:].rearrange("p (bh j d) -> p bh j d", bh=BH, j=8),
                    d_va[:].bitcast(f32r).rearrange("bh (j p) d -> p bh j d", p=128))
                tri = plc.tile([HD, 4 * 512], f32r)
                nc.gpsimd.dma_start(tri[:], d_tri[:].bitcast(f32r))
                idn = plc.tile([HD, HD], f32r)
                nc.gpsimd.dma_start(idn[:], d_idn[:].bitcast(f32r))
                bd = plc.tile([1, H], f32r)
                nc.gpsimd.dma_start(bd[:], d_bd[:].bitcast(f32r))

                wd_tiles = []
                for kb in range(KB):
                    wdt = pwd.tile([128, H], f32r, tag="wd", name=f"wd{kb}")
                    nc.gpsimd.dma_start(
                        wdt[:], d_wd[:].bitcast(f32r)[kb * 128:(kb + 1) * 128, :])
                    wd_tiles.append(wdt)

                # -------- attention per (b,h) --------
                with (
                    tc.tile_pool(name="sc", bufs=4) as psc,
                    tc.tile_pool(name="ps_sc", bufs=2, space="PSUM") as pssc,
                    tc.tile_pool(name="ps_tp", bufs=2, space="PSUM") as pstp,
                    tc.tile_pool(name="ps_at", bufs=2, space="PSUM") as psat,
                    tc.tile_pool(name="ps_rs", bufs=2, space="PSUM") as psrs,
                    tc.tile_pool(name="nrm", bufs=2) as pnrm,
                    tc.tile_pool(name="natout", bufs=1) as pnat,
                ):
                    for bh in range(BH):
                        b, hh = bh // HPC, bh % HPC
                        qsl = qk[:, hh * TOK + b * S: hh * TOK + b * S + S]
                        ksl = qk[:, (2 + hh) * TOK + b * S:
                                 (2 + hh) * TOK + b * S + S]
                        for qh in range(2):
                            kbs = list(range(8)) + [8 + j for j in range(8)
                                                    if 128 * j < 512 * (qh + 1)]
                            aps = psat.tile([128, 512], f32, tag="at",
                                            name=f"aps{bh}{qh}")
                            rps = psrs.tile([1, 512], f32, tag="rs",
                                            name=f"rps{bh}{qh}")
                            for i, kb in enumerate(kbs):
                                first, last = (i == 0), (i == len(kbs) - 1)
                                if kb < 8:
                                    lhs_k = kcache[:, bh * CACHED + kb * 128:
                                                   bh * CACHED + kb * 128 + 128]
                                else:
                                    j = kb - 8
                                    lhs_k = ksl[:, j * 128:(j + 1) * 128]
                                sps = pssc.tile([128, 512], f32, tag="sc",
                                                name=f"sps{bh}{qh}{kb}")
                                nc.tensor.matmul(
                                    sps[:], lhs_k,
                                    qsl[:, qh * 512:(qh + 1) * 512],
                                    start=True, stop=True)
                                ex = psc.tile([128, 512], f32r, tag="ex",
                                              name=f"ex{bh}{qh}{kb}")
                                nc.scalar.activation(
                                    ex[:], sps[:],
                                    mybir.ActivationFunctionType.Exp, scale=SCALE)
                                if kb >= 8:
                                    j = kb - 8
                                    qc0 = 128 * j - 512 * qh
                                    if qc0 >= 0:
                                        v = qc0 // 128
                                        nc.vector.tensor_mul(
                                            ex[:], ex[:],
                                            tri[:, v * 512:(v + 1) * 512])
                                nc.tensor.matmul(rps[:], on128[:], ex[:],
                                                 start=first, stop=last)
                                if kb < 8:
                                    lhs_v = va[:, (bh * 8 + kb) * HD:
                                               (bh * 8 + kb) * HD + HD]
                                else:
                                    j = kb - 8
                                    vtb = b * 8 + j
                                    lhs_v = vsb[:, vtb * 2 * HD + hh * HD:
                                                vtb * 2 * HD + hh * HD + HD]
                                nc.tensor.matmul(aps[:], lhs_v, ex[:],
                                                 start=first, stop=last)
                            # normalize: anT[:, q] = aps * (1/rsum)
                            rcp = pnrm.tile([1, 512], f32r, tag="rcp",
                                            name=f"rcp{bh}{qh}")
                            nc.vector.reciprocal(rcp[:], rps[:])
                            bcp = pssc.tile([128, 512], f32, tag="sc",
                                            name=f"bcp{bh}{qh}")
                            nc.tensor.matmul(bcp[:], on1[:], rcp[:],
                                             start=True, stop=True)
                            bcs = pnrm.tile([128, 512], f32, tag="bcs",
                                            name=f"bcs{bh}{qh}")
                            nc.vector.tensor_copy(bcs[:], bcp[:])
                            nc.vector.tensor_mul(
                                anT[:, bh * S + qh * 512:
                                    bh * S + qh * 512 + 512],
                                aps[:], bcs[:])

                        # ---- natural-layout outputs via PE transpose ----
                        kn = pnat.tile([128, S], f32, tag="kn", name=f"kn{bh}")
                        an = pnat.tile([128, S], f32, tag="an", name=f"an{bh}")
                        for tb in range(8):
                            tp = pstp.tile([128, 128], f32r, tag="tp",
                                           name=f"tpk{bh}{tb}")
                            nc.tensor.transpose(
                                tp[:], ksl[:, tb * 128:(tb + 1) * 128], idn[:])
                            nc.vector.tensor_copy(
                                kn[:, tb * 128:(tb + 1) * 128], tp[:])
                            tp2 = pstp.tile([128, 128], f32r, tag="tp",
                                            name=f"tpa{bh}{tb}")
                            nc.tensor.transpose(
                                tp2[:], anT[:, bh * S + tb * 128:
                                            bh * S + tb * 128 + 128], idn[:])
                            nc.vector.tensor_copy(
                                an[:, tb * 128:(tb + 1) * 128], tp2[:])
                        nc.sync.dma_start(
                            d_knew[bh].rearrange("(tb p) d -> p tb d", p=128),
                            kn[:].rearrange("p (tb d) -> p tb d", tb=8))
                        nc.sync.dma_start(
                            d_anew[bh].rearrange("(tb p) d -> p tb d", p=128),
                            an[:].rearrange("p (tb d) -> p tb d", tb=8))

                # -------- AllToAll --------
                a2a_i = pdram.tile([NC * 2 * HD, TSL], f32)
                for bh in range(BH):
                    b, hh = bh // HPC, bh % HPC
                    nc.sync.dma_start(
                        a2a_i.rearrange("(dest r) t -> r dest t", r=2 * HD)
                        [hh * HD:(hh + 1) * HD, 4 * b:4 * b + 4, :].bitcast(f32r),
                        anT[:, bh * S: (bh + 1) * S].rearrange(
                            "p (dd t) -> p dd t", dd=4))
                a2a_o = pdram.tile([NC * 2 * HD, TSL], f32)
                nc.gpsimd.collective_compute(
                    "AllToAll", mybir.AluOpType.bypass,
                    replica_groups=[list(range(NC))],
                    ins=[a2a_i.opt()], outs=[a2a_o.opt()])

                atall = pat.tile([128, KB * TSL], f32r)
                nc.sync.dma_start(
                    atall[:].rearrange("p (kb t) -> p kb t", kb=KB),
                    a2a_o.bitcast(f32r).rearrange("(kb p) t -> p kb t", p=128))

                # -------- dense projection for my 256-token slice --------
                with (
                    tc.tile_pool(name="ps_d", bufs=8, space="PSUM") as psd,
                    tc.tile_pool(name="od", bufs=2) as pod,
                ):
                    dps = {}
                    for tb in range(2):
                        for fb in range(4):
                            dps[(tb, fb)] = psd.tile(
                                [128, 512], f32, tag="d", name=f"dps_{tb}_{fb}")
                    for kb in range(KB):
                        for tb in range(2):
                            for fb in range(4):
                                nc.tensor.matmul(
                                    dps[(tb, fb)][:],
                                    atall[:, kb * TSL + tb * 128:
                                          kb * TSL + tb * 128 + 128],
                                    wd_tiles[kb][:, fb * 512:(fb + 1) * 512],
                                    start=(kb == 0), stop=False)
                    for tb in range(2):
                        od = pod.tile([128, H], f32, tag="od", name=f"od{tb}")
                        for fb in range(4):
                            nc.tensor.matmul(
                                dps[(tb, fb)][:], on1[:],
                                bd[:, fb * 512:(fb + 1) * 512],
                                start=False, stop=True)
                            nc.vector.tensor_copy(
                                od[:, fb * 512:(fb + 1) * 512], dps[(tb, fb)][:])
                        nc.sync.dma_start(d_out[tb * 128:(tb + 1) * 128, :], od[:])

    _split_multiwaits(nc)
    return nc


def _rope_tables(position_ids):
    pos = np.asarray(position_ids).astype(np.float64).reshape(TOK)
    inv_freq = 1.0 / (BASE ** (np.arange(0, ROT, 2, dtype=np.float64) / ROT))
    ang = pos[:, None] * inv_freq[None, :]          # [2048, 16]
    cos = np.cos(ang).astype(np.float32)
    sin = np.sin(ang).astype(np.float32)
    cs = np.empty((2 * ROT, TOK), dtype=np.float32)
    cs[0:16] = cos.T
    cs[16:32] = cos.T
    cs[32:48] = -sin.T
    cs[48:64] = sin.T
    return cs


def kernel(hidden_states, position_ids, cached_k, cached_a, W_qkv, b_qkv,
           W_dense, b_dense):
    global _COMPILED
    hidden_states = np.ascontiguousarray(np.asarray(hidden_states, dtype=np.float32))
    cached_k = np.asarray(cached_k, dtype=np.float32)
    cached_a = np.asarray(cached_a, dtype=np.float32)
    W_qkv = np.asarray(W_qkv, dtype=np.float32)
    b_qkv = np.asarray(b_qkv, dtype=np.float32)
    W_dense = np.ascontiguousarray(np.asarray(W_dense, dtype=np.float32))
    b_dense = np.asarray(b_dense, dtype=np.float32)

    if _COMPILED is None:
        _COMPILED = _build()
    nc = _COMPILED

    hiddenT = np.ascontiguousarray(hidden_states.reshape(TOK, H).T)
    cs = _rope_tables(position_ids)
    kk = np.arange(HD)[:, None]
    qq = np.arange(512)[None, :]
    tri = np.empty((HD, 4 * 512), dtype=np.float32)
    for v in range(4):
        qc0 = 128 * v
        tri[:, v * 512:(v + 1) * 512] = (
            (qq >= qc0 + 128) | ((qq >= qc0) & (kk <= qq - qc0))
        ).astype(np.float32)
    idn = np.eye(HD, dtype=np.float32)
    on128 = np.ones((HD, 1), dtype=np.float32)
    on1 = np.ones((1, HD), dtype=np.float32)
    bd = np.ascontiguousarray(b_dense.reshape(1, H))

    in_maps = []
    for c in range(NC):
        h0 = HPC * c
        cols = []
        for h in (h0, h0 + 1):
            cols.append(W_qkv[:, h * HD:(h + 1) * HD])           # q
        for h in (h0, h0 + 1):
            cols.append(W_qkv[:, H + h * HD: H + (h + 1) * HD])  # k
        wqk = np.ascontiguousarray(np.concatenate(cols, axis=1))
        wv = np.ascontiguousarray(
            W_qkv[:, 2 * H + h0 * HD: 2 * H + (h0 + 2) * HD])
        bqk = np.stack([b_qkv[h0 * HD:(h0 + 1) * HD],
                        b_qkv[(h0 + 1) * HD:(h0 + 2) * HD],
                        b_qkv[H + h0 * HD: H + (h0 + 1) * HD],
                        b_qkv[H + (h0 + 1) * HD: H + (h0 + 2) * HD]], axis=1)
        bv = np.ascontiguousarray(
            b_qkv[2 * H + h0 * HD: 2 * H + (h0 + 2) * HD].reshape(1, 2 * HD))
        kc = np.ascontiguousarray(
            cached_k[:, h0:h0 + 2].transpose(0, 1, 3, 2).reshape(BH, HD, CACHED))
        va = np.ascontiguousarray(cached_a[:, h0:h0 + 2].reshape(BH, CACHED, HD))
        in_maps.append(dict(
            ht=hiddenT, wqk=wqk, wv=wv, bqk=np.ascontiguousarray(bqk), bv=bv,
            kc=kc, va=va, wd=W_dense, bd=bd, cs=cs, tri=tri, idn=idn,
            on128=on128, on1=on1))

    res = run_bass_kernel_spmd(nc, in_maps, core_ids=list(range(NC)), trace=False)

    # ---- reassemble (concat only) ----
    output = np.empty((TOK, H), dtype=np.float32)
    full_key = np.empty((B, NH, T, HD), dtype=np.float32)
    new_cached_a = np.empty((B, NH, T, HD), dtype=np.float32)
    full_key[:, :, :CACHED] = cached_k
    new_cached_a[:, :, :CACHED] = cached_a
    for c in range(NC):
        h0 = HPC * c
        r = res.results[c]
        output[c * TSL:(c + 1) * TSL] = r["outp"]
        kn = r["knew"].reshape(B, HPC, S, HD)
        an = r["anew"].reshape(B, HPC, S, HD)
        full_key[:, h0:h0 + 2, CACHED:] = kn
        new_cached_a[:, h0:h0 + 2, CACHED:] = an
    return (output.reshape(B, S, H), full_key, new_cached_a)
